# revision 36
# baseline (speedup 1.0000x reference)
"""Trainium2 Bass kernel for nn_Attention_17532056502607.

Multi-head self-attention (B=8, N=48*48=2304 tokens, C=384, 8 heads of 48):
    q = x @ q_w.T + q_b ; k,v = x @ kv_w.T + kv_b
    out = softmax(q k^T / sqrt(48)) v ; y = out @ proj_w.T + proj_b

Sharding: data-parallel, one batch element per NeuronCore (8 cores).

Per-core algorithm (all in "S^T layout", keys on partitions — no transposes):
  - host supplies xT = x_b^T [C, N] and head-PAIR-packed weights: heads 2p /
    2p+1 of a pair live at partition rows 0-47 / 64-111, so two K=48 matmuls
    run concurrently in the PE array (row/col 32-tiles).
  - qT/kT [C_pair, N] = wT-pair @ xT          (PE, K=C=384)
  - v    [N, 8*49]    = x @ wv + rank-1 bias matmul; each head's V block is
    [ones | v0..v47], so attn@V also accumulates the softmax denominator at
    a 32-aligned output partition (0 / 64).
  - S^T  [keys, q]    = kT-tile.T @ qT        (K=48, row-packed head pairs)
  - expS = exp(scale * S^T)                   (ACT, reads PSUM, writes SBUF)
  - outT [49x2, q]   += (1|v).T @ expS        (K=128 keys, col strips 0/64)
  - normalize: drain to SBUF, exact reciprocal of rows 0/64, rank-1 selector
    matmul broadcasts the recips, DVE multiply.
  - y    [N, C]       = sum_pairs outT-pair.T @ projw-pair + bias, with K=113
    spanning both head blocks and zero weight rows under the denominators.

Matmul dtypes default to float32r for x->q/k/v and the output projection and
bf16 for the attention core (rel err ~2.7e-3 vs the fp32 reference; set
ATTN_MM_DT=float32 for exact-but-slow).
"""

import os
import sys

import numpy as np

for _p in ("/opt/trn_rl_repo",):
    if _p not in sys.path:
        sys.path.append(_p)

import concourse.bass as bass  # noqa: E402
import concourse.tile as tile  # noqa: E402
from concourse import bacc, mybir  # noqa: E402
from concourse.bass_utils import run_bass_kernel_spmd  # noqa: E402

# ---------------------------------------------------------------- constants
B = 8
HH = 48
WW = 48
C = 384
N = HH * WW  # 2304
NH = 8
HD = 48
PAIRS = NH // 2  # 4
P = 128
NT = N // P  # 18 token tiles
KTC = C // P  # 3 contraction tiles over C
SCALE = float(HD) ** -0.5
VW = NH * (HD + 1)  # 392: v with a ones column per head
CHUNKS = [(0, 512), (512, 512), (1024, 512), (1536, 512), (2048, 256)]

F32 = mybir.dt.float32
# Matmul dtype for all SBUF operands. float32 = exact but 4 cyc/row on PE;
# float32r = same bits, reduced-precision single-pass matmul (1 cyc/row for
# moving dim >= 256) but cannot write PSUM at partition base 64; bfloat16
# halves SBUF/DMA and enables FWL.
MM_DT = getattr(mybir.dt, os.environ.get("ATTN_MM_DT", "float32r"))
# attn@V runs in bf16 when MM_DT is float32r (f32r matmuls cannot col-tile to
# partition base 64; bf16 error here is averaged over the 2304-key softmax).
AV_DT = (
    mybir.dt.bfloat16
    if MM_DT == mybir.dt.float32r
    else getattr(mybir.dt, os.environ.get("ATTN_AV_DT", MM_DT.value))
)

# S^T (q@k) operand dtype. bf16 emits separate LDWEIGHTS instructions that
# overlap prior matmuls in the other row group (fp32r self-loads weights
# serially); the softmax averages away the extra rounding (+6% rel err).
ST_DT = getattr(
    mybir.dt,
    os.environ.get(
        "ATTN_ST_DT",
        "bfloat16" if MM_DT == mybir.dt.float32r else MM_DT.value,
    ),
)

# broadcast-matmul operand dtype: f32r is 4x faster on PE and legal at dst
# base 0; producers must write f32r-typed outputs (verifier checks rounding)
BC_DT = mybir.dt.float32r if MM_DT != mybir.dt.float32 else F32

_EXP = mybir.ActivationFunctionType.Exp


def _emit(tc: tile.TileContext, d: dict, ctx):
    nc = tc.nc

    persist = ctx.enter_context(tc.tile_pool(name="persist", bufs=1))
    v_sb = persist.tile([P, NT, VW], AV_DT, name="v_sb")
    qT_sb = persist.tile([P, PAIRS, N], ST_DT, name="qT_sb")
    kT_sb = persist.tile([P, PAIRS, N], ST_DT, name="kT_sb")
    oT_sb = persist.tile([P, PAIRS, N], MM_DT, name="oT_sb")
    pw_sb = persist.tile([P, PAIRS, C], MM_DT, name="pw_sb")
    qb_sb = persist.tile([P, PAIRS], F32, name="qb_sb")
    kb_sb = persist.tile([P, PAIRS], F32, name="kb_sb")
    vb_sb = persist.tile([1, VW], MM_DT, name="vb_sb")
    pb_sb = persist.tile([1, C], MM_DT, name="pb_sb")
    # fp32 ones vector (memset can't encode float32r); bitcast where an
    # MM_DT-typed operand is required — the bit pattern is identical.
    ones32 = persist.tile([1, P], F32, name="ones32")

    nc.sync.dma_start(pw_sb[:], d["pwP"].rearrange("r p m -> p r m"))
    nc.sync.dma_start(qb_sb[:], d["qbP"])
    nc.sync.dma_start(kb_sb[:], d["kbP"])
    nc.sync.dma_start(vb_sb[:], d["vbA"])
    nc.sync.dma_start(pb_sb[:], d["pbR"])
    selE_sb = persist.tile([1, 256], BC_DT, name="selE_sb")
    nc.sync.dma_start(selE_sb[:], d["selE"])
    nc.vector.memset(ones32[:], 1.0)
    # zero via an F32 view: memset can't encode float32r, but 0.0 is all-zero
    # bits in every dtype
    _oT_z = oT_sb[:] if MM_DT != mybir.dt.float32r else oT_sb[:].bitcast(F32)
    nc.vector.memset(_oT_z, 0.0)
    if MM_DT == mybir.dt.bfloat16:
        ones_mm = persist.tile([1, P], MM_DT, name="ones_mm")
        nc.vector.memset(ones_mm[:], 1.0)
    elif MM_DT == mybir.dt.float32r:
        ones_mm = ones32.bitcast(MM_DT)
    else:
        ones_mm = ones32

    # ---------------- phase A: projections q^T, k^T, v ----------------
    with (
        tc.tile_pool(name="phA", bufs=1) as phA,
        tc.tile_pool(name="psA", bufs=4, space="PSUM") as psA,
    ):
        xT_sb = phA.tile([P, KTC, N], MM_DT, name="xT_sb")
        wq_sb = phA.tile([P, KTC, PAIRS * P], MM_DT, name="wq_sb")
        wk_sb = phA.tile([P, KTC, PAIRS * P], MM_DT, name="wk_sb")
        wv_sb = phA.tile([P, KTC, VW], MM_DT, name="wv_sb")
        nc.sync.dma_start(xT_sb[:], d["xT"].rearrange("(kt p) n -> p kt n", p=P))
        nc.sync.dma_start(wq_sb[:], d["wqP"].rearrange("(kt p) m -> p kt m", p=P))
        nc.sync.dma_start(wk_sb[:], d["wkP"].rearrange("(kt p) m -> p kt m", p=P))
        nc.sync.dma_start(wv_sb[:], d["wvA"].rearrange("(kt p) m -> p kt m", p=P))

        # v natural [token, 8*(hd|1)]: K=C matmul + rank-1 (ones x vb_aug)
        # which adds the v bias AND writes 1.0 into each head's 49th column.
        for nt in range(NT):
            psv = psA.tile([P, 512], F32, name="psv", tag="psA")
            for kt in range(KTC):
                nc.tensor.matmul(
                    psv[:, 0:VW],
                    lhsT=xT_sb[:, kt, nt * P : (nt + 1) * P],
                    rhs=wv_sb[:, kt, :],
                    start=(kt == 0),
                    stop=False,
                )
            nc.tensor.matmul(
                psv[:, 0:VW],
                lhsT=ones_mm[:, 0:P],
                rhs=vb_sb[:],
                start=False,
                stop=True,
            )
            nc.vector.tensor_copy(v_sb[:, nt, :], psv[:, 0:VW])

        # q^T/k^T in pair layout: out partitions = pair block of C_out.
        for pr in range(PAIRS):
            for q0, qw in CHUNKS:
                psq = psA.tile([P, 512], F32, name="psq", tag="psA")
                psk = psA.tile([P, 512], F32, name="psk", tag="psA")
                for kt in range(KTC):
                    nc.tensor.matmul(
                        psq[:, 0:qw],
                        lhsT=wq_sb[:, kt, pr * P : (pr + 1) * P],
                        rhs=xT_sb[:, kt, q0 : q0 + qw],
                        start=(kt == 0),
                        stop=(kt == KTC - 1),
                    )
                    nc.tensor.matmul(
                        psk[:, 0:qw],
                        lhsT=wk_sb[:, kt, pr * P : (pr + 1) * P],
                        rhs=xT_sb[:, kt, q0 : q0 + qw],
                        start=(kt == 0),
                        stop=(kt == KTC - 1),
                    )
                nc.vector.tensor_scalar_add(
                    qT_sb[:, pr, q0 : q0 + qw], psq[:, 0:qw], qb_sb[:, pr : pr + 1]
                )
                nc.vector.tensor_scalar_add(
                    kT_sb[:, pr, q0 : q0 + qw], psk[:, 0:qw], kb_sb[:, pr : pr + 1]
                )

    # ---------------- attention: flash over q chunks, S^T layout -------
    _nb = 2 if MM_DT == mybir.dt.float32 else 4
    with (
        tc.tile_pool(name="es", bufs=_nb + 1) as es_pool,
        tc.tile_pool(name="rcp", bufs=_nb) as rc_pool,
        tc.tile_pool(name="psS", bufs=1, space="PSUM") as psS,
        tc.tile_pool(name="psO", bufs=1, space="PSUM") as psO,
        tc.tile_pool(name="psB", bufs=1, space="PSUM") as psB,
    ):
        # alternating 3-slot/2-slot score groups: two tags of one buf each act
        # as a double buffer in 5 banks, leaving a dedicated bank for bc
        GSIZES = [3, 2] * 6 + [3, 3]
        # den tiles seed the batched reciprocal; rows 1-63 stay at 1.0 so a
        # single [0:65] reciprocal never sees junk
        den_tiles = [
            rc_pool.tile([P, 512], F32, name=f"den{i}", tag=f"den{i}")
            for i in range(2)
        ]
        for dt_ in den_tiles:
            nc.vector.memset(dt_[:], 1.0)
        pending = None
        ci = 0
        for pr in range(PAIRS):
            for q0, qw in CHUNKS:
                # separate accumulator banks per head; col strips 0-1 vs 2-3
                # (out base partition 0 vs 64) keep the two matmuls concurrent
                oTA = psO.tile([P, 512], F32, name="oTA", tag="oTA")
                oTB = psO.tile([P, 512], F32, name="oTB", tag="oTB")
                seq = [(kt, hoff) for kt in range(NT) for hoff in (0, 64)]

                def attnv(est, si, gs, oTA=oTA, oTB=oTB, pr=pr, qw=qw):
                    for j in range(gs):
                        kt2, hoff2 = seq[si + j]
                        h = pr * 2 + (0 if hoff2 == 0 else 1)
                        oT = oTA if hoff2 == 0 else oTB
                        nc.tensor.matmul(
                            oT[hoff2 : hoff2 + HD + 1, 0:qw],
                            lhsT=v_sb[:, kt2, h * (HD + 1) : (h + 1) * (HD + 1)],
                            rhs=est[:, j, 0:qw],
                            start=(kt2 == 0),
                            stop=(kt2 == NT - 1),
                        )

                si = 0
                av_q = []
                for gi, gs in enumerate(GSIZES):
                    if gi == 8 and pending is not None:
                        # emit the previous chunk's normalize here, well after
                        # its reciprocal has drained, so the PE never stalls
                        # on it at the chunk boundary
                        pending()
                        pending = None
                    sg = psS.tile([P, gs, 512], F32, name="sg", tag=f"sg{gs}")
                    for j in range(gs):
                        kt, hoff = seq[si + j]
                        nc.tensor.matmul(
                            sg[:, j, 0:qw],
                            lhsT=kT_sb[hoff : hoff + HD, pr, kt * P : (kt + 1) * P],
                            rhs=qT_sb[hoff : hoff + HD, pr, q0 : q0 + qw],
                            start=True,
                            stop=True,
                        )
                    est = es_pool.tile(
                        [P, gs, 512], AV_DT, name="est", tag=f"est{gs}"
                    )
                    nc.scalar.activation(
                        est[:, :, 0:qw], sg[:, :, 0:qw], _EXP, scale=SCALE
                    )
                    # emit attn@V two groups late: the PE stream then runs
                    # S^T(g) while ACT computes exp(g-2)/exp(g-1), instead of
                    # stalling in-order on exp latency
                    av_q.append((est, si, gs))
                    if len(av_q) > 3:
                        attnv(*av_q.pop(0))
                    si += gs
                for av in av_q:
                    attnv(*av)
                # drain the accumulators to SBUF right away: these DVE copies
                # are emitted BEFORE the previous chunk's normalize chain so
                # the in-order DVE queue frees the oT banks first. The den
                # copies also seed the batched reciprocal (rows 0/64; rows
                # 1-63 of den stay 1.0 from the one-time memset).
                oU = rc_pool.tile([P, 512], F32, name="oU", tag="oU")
                den = den_tiles[ci % 2]
                nc.vector.tensor_copy(oU[0 : HD + 1, 0:qw], oTA[0 : HD + 1, 0:qw])
                nc.vector.tensor_copy(
                    oU[64 : 64 + HD + 1, 0:qw], oTB[64 : 64 + HD + 1, 0:qw]
                )
                # den copies go to ScalarE so both oT-draining engines run
                # in parallel and the banks free ~2x sooner for the next chunk
                nc.scalar.copy(den[0:1, 0:qw], oTA[0:1, 0:qw])
                nc.scalar.copy(den[64:65, 0:qw], oTB[64:65, 0:qw])

                # one reciprocal covers both denominators (rows 0 and 64);
                # emitted eagerly so DVE computes it during the next chunk
                rec = rc_pool.tile([P, 512], BC_DT, name="rec", tag="rc")
                with nc.allow_low_precision(
                    reason="float32r keeps fp32 bits; PE rounds on read"
                ):
                    nc.vector.reciprocal(rec[0:65, 0:qw], den[0:65, 0:qw])

                def normalize(pr=pr, q0=q0, qw=qw, oU=oU, rec=rec):
                    # copy recipB down to a base-0 row so both selector
                    # matmuls keep base partition 0
                    rec1 = rc_pool.tile([1, 512], BC_DT, name="rec1", tag="rc1")
                    nc.vector.tensor_copy(rec1[0:1, 0:qw], rec[64:65, 0:qw])
                    # rank-1 selector matmuls broadcast recipA to bc rows
                    # 0-48 and recipB to rows 64-112
                    bc = psB.tile([P, 512], F32, name="bc", tag="bc")
                    nc.tensor.matmul(
                        bc[0:113, 0:qw],
                        lhsT=selE_sb[0:1, 0:113],
                        rhs=rec[0:1, 0:qw],
                        start=True, stop=False,
                    )
                    nc.tensor.matmul(
                        bc[0:113, 0:qw],
                        lhsT=selE_sb[0:1, 128:241],
                        rhs=rec1[0:1, 0:qw],
                        start=False, stop=True,
                    )
                    bcs = rc_pool.tile([P, 512], F32, name="bcs", tag="bcs")
                    nc.vector.tensor_copy(bcs[0:113, 0:qw], bc[0:113, 0:qw])
                    nc.vector.tensor_mul(
                        oT_sb[0 : HD + 1, pr, q0 : q0 + qw],
                        oU[0 : HD + 1, 0:qw],
                        bcs[0 : HD + 1, 0:qw],
                    )
                    nc.vector.tensor_mul(
                        oT_sb[64 : 64 + HD + 1, pr, q0 : q0 + qw],
                        oU[64 : 64 + HD + 1, 0:qw],
                        bcs[64 : 64 + HD + 1, 0:qw],
                    )

                pending = normalize
                ci += 1
        if pending is not None:
            pending()

    # ---------------- output projection ---------------------------------
    with (
        tc.tile_pool(name="fin", bufs=3) as fin_pool,
        tc.tile_pool(name="psF", bufs=4, space="PSUM") as psF,
    ):
        for nt in range(NT):
            fF = psF.tile([P, C], F32, name="fF", tag="f")
            for pr in range(PAIRS):
                # K=113 spans both heads; pw rows 0, 49-63, 64 are zero and
                # oT_sb rows 49-63 are zeroed once, so denom rows drop out.
                nc.tensor.matmul(
                    fF[:],
                    lhsT=oT_sb[0:113, pr, nt * P : (nt + 1) * P],
                    rhs=pw_sb[0:113, pr, :],
                    start=(pr == 0),
                    stop=False,
                )
            nc.tensor.matmul(
                fF[:], lhsT=ones_mm[:, 0:P], rhs=pb_sb[:], start=False, stop=True
            )
            ft = fin_pool.tile([P, C], F32, name="ft", tag="ft")
            nc.vector.tensor_copy(ft[:], fF[:])
            nc.sync.dma_start(d["out"][nt * P : (nt + 1) * P, :], ft[:])


def build_program(n_cores: int = 8):
    nc = bacc.Bacc(
        "TRN2",
        target_bir_lowering=False,
        debug=False,
        enable_asserts=False,
        num_devices=n_cores,
    )
    d = {
        "xT": nc.dram_tensor("xT", [C, N], MM_DT, kind="ExternalInput").ap(),
        "wqP": nc.dram_tensor("wqP", [C, PAIRS * P], MM_DT, kind="ExternalInput").ap(),
        "wkP": nc.dram_tensor("wkP", [C, PAIRS * P], MM_DT, kind="ExternalInput").ap(),
        "wvA": nc.dram_tensor("wvA", [C, VW], MM_DT, kind="ExternalInput").ap(),
        "vbA": nc.dram_tensor("vbA", [1, VW], MM_DT, kind="ExternalInput").ap(),
        "qbP": nc.dram_tensor("qbP", [P, PAIRS], F32, kind="ExternalInput").ap(),
        "kbP": nc.dram_tensor("kbP", [P, PAIRS], F32, kind="ExternalInput").ap(),
        "pwP": nc.dram_tensor("pwP", [PAIRS, P, C], MM_DT, kind="ExternalInput").ap(),
        "pbR": nc.dram_tensor("pbR", [1, C], MM_DT, kind="ExternalInput").ap(),
        "selE": nc.dram_tensor("selE", [1, 256], BC_DT, kind="ExternalInput").ap(),
        "out": nc.dram_tensor("out", [N, C], F32, kind="ExternalOutput").ap(),
    }
    import contextlib

    with tile.TileContext(nc) as tc:
        with contextlib.ExitStack() as ctx:
            _emit(tc, d, ctx)
    nc.finalize()
    return nc


def _mm_np_dtype():
    if MM_DT == mybir.dt.bfloat16:
        import ml_dtypes

        return ml_dtypes.bfloat16
    return np.float32


def _prep_host(x, q_w, q_b, kv_w, kv_b, proj_w, proj_b):
    """Transpose/pack on host. Returns (per-core xT list, shared map)."""
    f32 = np.float32
    x = np.asarray(x, f32)
    xT = np.ascontiguousarray(x.reshape(B, N, C).transpose(0, 2, 1))  # [B, C, N]

    qwT = np.ascontiguousarray(np.asarray(q_w, f32).T)  # [Cin, Cout]
    kwT = np.ascontiguousarray(np.asarray(kv_w[:C], f32).T)
    vwT = np.ascontiguousarray(np.asarray(kv_w[C:], f32).T)
    pwT = np.ascontiguousarray(np.asarray(proj_w, f32).T)

    wqP = np.zeros((C, PAIRS * P), f32)
    wkP = np.zeros((C, PAIRS * P), f32)
    qbP = np.zeros((P, PAIRS), f32)
    kbP = np.zeros((P, PAIRS), f32)
    pwP = np.zeros((PAIRS, P, C), f32)
    for p in range(PAIRS):
        a, b = 2 * p, 2 * p + 1
        wqP[:, p * P : p * P + HD] = qwT[:, a * HD : (a + 1) * HD]
        wqP[:, p * P + 64 : p * P + 64 + HD] = qwT[:, b * HD : (b + 1) * HD]
        wkP[:, p * P : p * P + HD] = kwT[:, a * HD : (a + 1) * HD]
        wkP[:, p * P + 64 : p * P + 64 + HD] = kwT[:, b * HD : (b + 1) * HD]
        qbP[0:HD, p] = q_b[a * HD : (a + 1) * HD]
        qbP[64 : 64 + HD, p] = q_b[b * HD : (b + 1) * HD]
        kbP[0:HD, p] = kv_b[a * HD : (a + 1) * HD]
        kbP[64 : 64 + HD, p] = kv_b[b * HD : (b + 1) * HD]
        # rows 1..48 / 65..112 carry the proj weights; rows 0 / 64 stay zero
        # to swallow the denominator row of outT.
        pwP[p, 1 : 1 + HD, :] = pwT[a * HD : (a + 1) * HD, :]
        pwP[p, 65 : 65 + HD, :] = pwT[b * HD : (b + 1) * HD, :]

    # V blocks are [ones | v0..v47] per head so the softmax denominator lands
    # at a 32-aligned PSUM partition (0 / 64).
    wvA = np.zeros((C, VW), f32)
    vbA = np.zeros((1, VW), f32)
    for h in range(NH):
        wvA[:, h * (HD + 1) + 1 : (h + 1) * (HD + 1)] = vwT[:, h * HD : (h + 1) * HD]
        vbA[0, h * (HD + 1) + 1 : (h + 1) * (HD + 1)] = kv_b[
            C + h * HD : C + (h + 1) * HD
        ]
        vbA[0, h * (HD + 1)] = 1.0

    selE = np.zeros((1, 256), f32)
    selE[0, 0 : HD + 1] = 1.0
    selE[0, 128 + 64 : 128 + 64 + HD + 1] = 1.0

    mmdt = _mm_np_dtype()
    shared = {
        "selE": selE,
        "wqP": wqP.astype(mmdt),
        "wkP": wkP.astype(mmdt),
        "wvA": wvA.astype(mmdt),
        "vbA": vbA.astype(mmdt),
        "qbP": qbP,
        "kbP": kbP,
        "pwP": pwP.astype(mmdt),
        "pbR": np.asarray(proj_b, f32).reshape(1, C).astype(mmdt),
    }
    return xT.astype(mmdt), shared


_PROGRAM = None


def _get_program():
    global _PROGRAM
    if _PROGRAM is None:
        _PROGRAM = build_program(B)
    return _PROGRAM


def kernel(x, q_w, q_b, kv_w, kv_b, proj_w, proj_b):
    xT, shared = _prep_host(x, q_w, q_b, kv_w, kv_b, proj_w, proj_b)
    nc = _get_program()
    in_maps = [dict(shared, xT=np.ascontiguousarray(xT[b])) for b in range(B)]
    res = run_bass_kernel_spmd(nc, in_maps, list(range(B)))
    outs = [np.asarray(res.results[i]["out"], np.float32) for i in range(B)]
    return np.stack(outs).reshape(B, HH, WW, C)


# revision 37
# speedup vs baseline: 1.0281x; 1.0281x over previous
"""Trainium2 Bass kernel for nn_Attention_17532056502607.

Multi-head self-attention (B=8, N=48*48=2304 tokens, C=384, 8 heads of 48):
    q = x @ q_w.T + q_b ; k,v = x @ kv_w.T + kv_b
    out = softmax(q k^T / sqrt(48)) v ; y = out @ proj_w.T + proj_b

Sharding: data-parallel, one batch element per NeuronCore (8 cores).

Per-core algorithm (all in "S^T layout", keys on partitions — no transposes):
  - host supplies xT = x_b^T [C, N] and head-PAIR-packed weights: heads 2p /
    2p+1 of a pair live at partition rows 0-47 / 64-111, so two K=48 matmuls
    run concurrently in the PE array (row/col 32-tiles).
  - qT/kT [C_pair, N] = wT-pair @ xT          (PE, K=C=384)
  - v    [N, 8*49]    = x @ wv + rank-1 bias matmul; each head's V block is
    [ones | v0..v47], so attn@V also accumulates the softmax denominator at
    a 32-aligned output partition (0 / 64).
  - S^T  [keys, q]    = kT-tile.T @ qT        (K=48, row-packed head pairs)
  - expS = exp(scale * S^T)                   (ACT, reads PSUM, writes SBUF)
  - outT [49x2, q]   += (1|v).T @ expS        (K=128 keys, col strips 0/64)
  - normalize: drain to SBUF, exact reciprocal of rows 0/64, rank-1 selector
    matmul broadcasts the recips, DVE multiply.
  - y    [N, C]       = sum_pairs outT-pair.T @ projw-pair + bias, with K=113
    spanning both head blocks and zero weight rows under the denominators.

Matmul dtypes default to float32r for x->q/k/v and the output projection and
bf16 for the attention core (rel err ~2.7e-3 vs the fp32 reference; set
ATTN_MM_DT=float32 for exact-but-slow).
"""

import os
import sys

import numpy as np

for _p in ("/opt/trn_rl_repo",):
    if _p not in sys.path:
        sys.path.append(_p)

import concourse.bass as bass  # noqa: E402
import concourse.tile as tile  # noqa: E402
from concourse import bacc, mybir  # noqa: E402
from concourse.bass_utils import run_bass_kernel_spmd  # noqa: E402

# ---------------------------------------------------------------- constants
B = 8
HH = 48
WW = 48
C = 384
N = HH * WW  # 2304
NH = 8
HD = 48
PAIRS = NH // 2  # 4
P = 128
NT = N // P  # 18 token tiles
KTC = C // P  # 3 contraction tiles over C
SCALE = float(HD) ** -0.5
VW = NH * (HD + 1)  # 392: v with a ones column per head
CHUNKS = [(0, 512), (512, 512), (1024, 512), (1536, 512), (2048, 256)]

F32 = mybir.dt.float32
# Matmul dtype for all SBUF operands. float32 = exact but 4 cyc/row on PE;
# float32r = same bits, reduced-precision single-pass matmul (1 cyc/row for
# moving dim >= 256) but cannot write PSUM at partition base 64; bfloat16
# halves SBUF/DMA and enables FWL.
MM_DT = getattr(mybir.dt, os.environ.get("ATTN_MM_DT", "float32r"))
# attn@V runs in bf16 when MM_DT is float32r (f32r matmuls cannot col-tile to
# partition base 64; bf16 error here is averaged over the 2304-key softmax).
AV_DT = (
    mybir.dt.bfloat16
    if MM_DT == mybir.dt.float32r
    else getattr(mybir.dt, os.environ.get("ATTN_AV_DT", MM_DT.value))
)

# S^T (q@k) operand dtype. bf16 emits separate LDWEIGHTS instructions that
# overlap prior matmuls in the other row group (fp32r self-loads weights
# serially); the softmax averages away the extra rounding (+6% rel err).
ST_DT = getattr(
    mybir.dt,
    os.environ.get(
        "ATTN_ST_DT",
        "bfloat16" if MM_DT == mybir.dt.float32r else MM_DT.value,
    ),
)

# broadcast-matmul operand dtype: f32r is 4x faster on PE and legal at dst
# base 0; producers must write f32r-typed outputs (verifier checks rounding)
BC_DT = mybir.dt.float32r if MM_DT != mybir.dt.float32 else F32

_EXP = mybir.ActivationFunctionType.Exp


def _emit(tc: tile.TileContext, d: dict, ctx):
    nc = tc.nc

    persist = ctx.enter_context(tc.tile_pool(name="persist", bufs=1))
    v_sb = persist.tile([P, NT, VW], AV_DT, name="v_sb")
    qT_sb = persist.tile([P, PAIRS, N], ST_DT, name="qT_sb")
    kT_sb = persist.tile([P, PAIRS, N], ST_DT, name="kT_sb")
    oT_sb = persist.tile([P, PAIRS, N], MM_DT, name="oT_sb")
    pw_sb = persist.tile([P, PAIRS, C], MM_DT, name="pw_sb")
    qb_sb = persist.tile([P, PAIRS], F32, name="qb_sb")
    kb_sb = persist.tile([P, PAIRS], F32, name="kb_sb")
    vb_sb = persist.tile([1, VW], MM_DT, name="vb_sb")
    pb_sb = persist.tile([1, C], MM_DT, name="pb_sb")
    # fp32 ones vector (memset can't encode float32r); bitcast where an
    # MM_DT-typed operand is required — the bit pattern is identical.
    ones32 = persist.tile([1, P], F32, name="ones32")

    nc.sync.dma_start(pw_sb[:], d["pwP"].rearrange("r p m -> p r m"))
    nc.sync.dma_start(qb_sb[:], d["qbP"])
    nc.sync.dma_start(kb_sb[:], d["kbP"])
    nc.sync.dma_start(vb_sb[:], d["vbA"])
    nc.sync.dma_start(pb_sb[:], d["pbR"])
    selE_sb = persist.tile([1, 256], BC_DT, name="selE_sb")
    nc.sync.dma_start(selE_sb[:], d["selE"])
    nc.vector.memset(ones32[:], 1.0)
    # zero via an F32 view: memset can't encode float32r, but 0.0 is all-zero
    # bits in every dtype
    _oT_z = oT_sb[:] if MM_DT != mybir.dt.float32r else oT_sb[:].bitcast(F32)
    nc.vector.memset(_oT_z, 0.0)
    if MM_DT == mybir.dt.bfloat16:
        ones_mm = persist.tile([1, P], MM_DT, name="ones_mm")
        nc.vector.memset(ones_mm[:], 1.0)
    elif MM_DT == mybir.dt.float32r:
        ones_mm = ones32.bitcast(MM_DT)
    else:
        ones_mm = ones32

    # ---------------- phase A: projections q^T, k^T, v ----------------
    with (
        tc.tile_pool(name="phA", bufs=1) as phA,
        tc.tile_pool(name="psA", bufs=4, space="PSUM") as psA,
    ):
        xT_sb = phA.tile([P, KTC, N], MM_DT, name="xT_sb")
        wq_sb = phA.tile([P, KTC, PAIRS * P], MM_DT, name="wq_sb")
        wk_sb = phA.tile([P, KTC, PAIRS * P], MM_DT, name="wk_sb")
        wv_sb = phA.tile([P, KTC, VW], MM_DT, name="wv_sb")
        nc.sync.dma_start(xT_sb[:], d["xT"].rearrange("(kt p) n -> p kt n", p=P))
        nc.sync.dma_start(wq_sb[:], d["wqP"].rearrange("(kt p) m -> p kt m", p=P))
        nc.sync.dma_start(wk_sb[:], d["wkP"].rearrange("(kt p) m -> p kt m", p=P))
        nc.sync.dma_start(wv_sb[:], d["wvA"].rearrange("(kt p) m -> p kt m", p=P))

        # v natural [token, 8*(hd|1)]: K=C matmul + rank-1 (ones x vb_aug)
        # which adds the v bias AND writes 1.0 into each head's 49th column.
        for nt in range(NT):
            psv = psA.tile([P, 512], F32, name="psv", tag="psA")
            for kt in range(KTC):
                nc.tensor.matmul(
                    psv[:, 0:VW],
                    lhsT=xT_sb[:, kt, nt * P : (nt + 1) * P],
                    rhs=wv_sb[:, kt, :],
                    start=(kt == 0),
                    stop=False,
                )
            nc.tensor.matmul(
                psv[:, 0:VW],
                lhsT=ones_mm[:, 0:P],
                rhs=vb_sb[:],
                start=False,
                stop=True,
            )
            nc.vector.tensor_copy(v_sb[:, nt, :], psv[:, 0:VW])

        # q^T/k^T in pair layout: out partitions = pair block of C_out.
        for pr in range(PAIRS):
            for q0, qw in CHUNKS:
                psq = psA.tile([P, 512], F32, name="psq", tag="psA")
                psk = psA.tile([P, 512], F32, name="psk", tag="psA")
                for kt in range(KTC):
                    nc.tensor.matmul(
                        psq[:, 0:qw],
                        lhsT=wq_sb[:, kt, pr * P : (pr + 1) * P],
                        rhs=xT_sb[:, kt, q0 : q0 + qw],
                        start=(kt == 0),
                        stop=(kt == KTC - 1),
                    )
                    nc.tensor.matmul(
                        psk[:, 0:qw],
                        lhsT=wk_sb[:, kt, pr * P : (pr + 1) * P],
                        rhs=xT_sb[:, kt, q0 : q0 + qw],
                        start=(kt == 0),
                        stop=(kt == KTC - 1),
                    )
                nc.vector.tensor_scalar_add(
                    qT_sb[:, pr, q0 : q0 + qw], psq[:, 0:qw], qb_sb[:, pr : pr + 1]
                )
                nc.vector.tensor_scalar_add(
                    kT_sb[:, pr, q0 : q0 + qw], psk[:, 0:qw], kb_sb[:, pr : pr + 1]
                )

    # ---------------- attention: flash over q chunks, S^T layout -------
    _nb = 2 if MM_DT == mybir.dt.float32 else 4
    with (
        tc.tile_pool(name="es", bufs=_nb + 1) as es_pool,
        tc.tile_pool(name="rcp", bufs=_nb) as rc_pool,
        tc.tile_pool(name="psS", bufs=1, space="PSUM") as psS,
        tc.tile_pool(name="psO", bufs=1, space="PSUM") as psO,
        tc.tile_pool(name="psB", bufs=1, space="PSUM") as psB,
    ):
        # alternating 3-slot/2-slot score groups: two tags of one buf each act
        # as a double buffer in 5 banks, leaving a dedicated bank for bc
        GSIZES = [3, 2] * 6 + [3, 3]
        # den tiles seed the batched reciprocal; rows 1-63 stay at 1.0 so a
        # single [0:65] reciprocal never sees junk
        den_tiles = [
            rc_pool.tile([P, 512], F32, name=f"den{i}", tag=f"den{i}")
            for i in range(2)
        ]
        for dt_ in den_tiles:
            nc.vector.memset(dt_[:], 1.0)
        pending = None
        ci = 0
        for pr in range(PAIRS):
            for q0, qw in CHUNKS:
                # separate accumulator banks per head; col strips 0-1 vs 2-3
                # (out base partition 0 vs 64) keep the two matmuls concurrent
                oTA = psO.tile([P, 512], F32, name="oTA", tag="oTA")
                oTB = psO.tile([P, 512], F32, name="oTB", tag="oTB")
                seq = [(kt, hoff) for kt in range(NT) for hoff in (0, 64)]

                def attnv(est, si, gs, oTA=oTA, oTB=oTB, pr=pr, qw=qw):
                    for j in range(gs):
                        kt2, hoff2 = seq[si + j]
                        h = pr * 2 + (0 if hoff2 == 0 else 1)
                        oT = oTA if hoff2 == 0 else oTB
                        nc.tensor.matmul(
                            oT[hoff2 : hoff2 + HD + 1, 0:qw],
                            lhsT=v_sb[:, kt2, h * (HD + 1) : (h + 1) * (HD + 1)],
                            rhs=est[:, j, 0:qw],
                            start=(kt2 == 0),
                            stop=(kt2 == NT - 1),
                        )

                si = 0
                av_q = []
                for gi, gs in enumerate(GSIZES):
                    if gi == 8 and pending is not None:
                        # emit the previous chunk's normalize here, well after
                        # its reciprocal has drained, so the PE never stalls
                        # on it at the chunk boundary
                        pending()
                        pending = None
                    sg = psS.tile([P, gs, 512], F32, name="sg", tag=f"sg{gs}")
                    for j in range(gs):
                        kt, hoff = seq[si + j]
                        nc.tensor.matmul(
                            sg[:, j, 0:qw],
                            lhsT=kT_sb[hoff : hoff + HD, pr, kt * P : (kt + 1) * P],
                            rhs=qT_sb[hoff : hoff + HD, pr, q0 : q0 + qw],
                            start=True,
                            stop=True,
                        )
                    est = es_pool.tile(
                        [P, gs, 512], AV_DT, name="est", tag=f"est{gs}"
                    )
                    nc.scalar.activation(
                        est[:, :, 0:qw], sg[:, :, 0:qw], _EXP, scale=SCALE
                    )
                    # emit attn@V two groups late: the PE stream then runs
                    # S^T(g) while ACT computes exp(g-2)/exp(g-1), instead of
                    # stalling in-order on exp latency
                    av_q.append((est, si, gs))
                    if len(av_q) > 3:
                        attnv(*av_q.pop(0))
                    si += gs
                for av in av_q:
                    attnv(*av)
                # drain the accumulators to SBUF right away: these DVE copies
                # are emitted BEFORE the previous chunk's normalize chain so
                # the in-order DVE queue frees the oT banks first. The den
                # copies also seed the batched reciprocal (rows 0/64; rows
                # 1-63 of den stay 1.0 from the one-time memset).
                oU = rc_pool.tile([P, 512], F32, name="oU", tag="oU")
                den = den_tiles[ci % 2]
                nc.vector.tensor_copy(oU[0 : HD + 1, 0:qw], oTA[0 : HD + 1, 0:qw])
                nc.vector.tensor_copy(
                    oU[64 : 64 + HD + 1, 0:qw], oTB[64 : 64 + HD + 1, 0:qw]
                )
                nc.vector.tensor_copy(den[0:1, 0:qw], oTA[0:1, 0:qw])
                nc.vector.tensor_copy(den[64:65, 0:qw], oTB[64:65, 0:qw])

                # one reciprocal covers both denominators (rows 0 and 64);
                # emitted eagerly so DVE computes it during the next chunk
                rec = rc_pool.tile([P, 512], BC_DT, name="rec", tag="rc")
                with nc.allow_low_precision(
                    reason="float32r keeps fp32 bits; PE rounds on read"
                ):
                    nc.vector.reciprocal(rec[0:65, 0:qw], den[0:65, 0:qw])

                def normalize(pr=pr, q0=q0, qw=qw, oU=oU, rec=rec):
                    # copy recipB down to a base-0 row so both selector
                    # matmuls keep base partition 0
                    rec1 = rc_pool.tile([1, 512], BC_DT, name="rec1", tag="rc1")
                    nc.vector.tensor_copy(rec1[0:1, 0:qw], rec[64:65, 0:qw])
                    # rank-1 selector matmuls broadcast recipA to bc rows
                    # 0-48 and recipB to rows 64-112
                    bc = psB.tile([P, 512], F32, name="bc", tag="bc")
                    nc.tensor.matmul(
                        bc[0:113, 0:qw],
                        lhsT=selE_sb[0:1, 0:113],
                        rhs=rec[0:1, 0:qw],
                        start=True, stop=False,
                    )
                    nc.tensor.matmul(
                        bc[0:113, 0:qw],
                        lhsT=selE_sb[0:1, 128:241],
                        rhs=rec1[0:1, 0:qw],
                        start=False, stop=True,
                    )
                    bcs = rc_pool.tile([P, 512], F32, name="bcs", tag="bcs")
                    nc.vector.tensor_copy(bcs[0:113, 0:qw], bc[0:113, 0:qw])
                    nc.vector.tensor_mul(
                        oT_sb[0 : HD + 1, pr, q0 : q0 + qw],
                        oU[0 : HD + 1, 0:qw],
                        bcs[0 : HD + 1, 0:qw],
                    )
                    nc.vector.tensor_mul(
                        oT_sb[64 : 64 + HD + 1, pr, q0 : q0 + qw],
                        oU[64 : 64 + HD + 1, 0:qw],
                        bcs[64 : 64 + HD + 1, 0:qw],
                    )

                pending = normalize
                ci += 1
        if pending is not None:
            pending()

    # ---------------- output projection ---------------------------------
    with (
        tc.tile_pool(name="fin", bufs=3) as fin_pool,
        tc.tile_pool(name="psF", bufs=4, space="PSUM") as psF,
    ):
        for nt in range(NT):
            fF = psF.tile([P, C], F32, name="fF", tag="f")
            for pr in range(PAIRS):
                # K=113 spans both heads; pw rows 0, 49-63, 64 are zero and
                # oT_sb rows 49-63 are zeroed once, so denom rows drop out.
                nc.tensor.matmul(
                    fF[:],
                    lhsT=oT_sb[0:113, pr, nt * P : (nt + 1) * P],
                    rhs=pw_sb[0:113, pr, :],
                    start=(pr == 0),
                    stop=False,
                )
            nc.tensor.matmul(
                fF[:], lhsT=ones_mm[:, 0:P], rhs=pb_sb[:], start=False, stop=True
            )
            ft = fin_pool.tile([P, C], F32, name="ft", tag="ft")
            nc.vector.tensor_copy(ft[:], fF[:])
            nc.sync.dma_start(d["out"][nt * P : (nt + 1) * P, :], ft[:])


def build_program(n_cores: int = 8):
    nc = bacc.Bacc(
        "TRN2",
        target_bir_lowering=False,
        debug=False,
        enable_asserts=False,
        num_devices=n_cores,
    )
    d = {
        "xT": nc.dram_tensor("xT", [C, N], MM_DT, kind="ExternalInput").ap(),
        "wqP": nc.dram_tensor("wqP", [C, PAIRS * P], MM_DT, kind="ExternalInput").ap(),
        "wkP": nc.dram_tensor("wkP", [C, PAIRS * P], MM_DT, kind="ExternalInput").ap(),
        "wvA": nc.dram_tensor("wvA", [C, VW], MM_DT, kind="ExternalInput").ap(),
        "vbA": nc.dram_tensor("vbA", [1, VW], MM_DT, kind="ExternalInput").ap(),
        "qbP": nc.dram_tensor("qbP", [P, PAIRS], F32, kind="ExternalInput").ap(),
        "kbP": nc.dram_tensor("kbP", [P, PAIRS], F32, kind="ExternalInput").ap(),
        "pwP": nc.dram_tensor("pwP", [PAIRS, P, C], MM_DT, kind="ExternalInput").ap(),
        "pbR": nc.dram_tensor("pbR", [1, C], MM_DT, kind="ExternalInput").ap(),
        "selE": nc.dram_tensor("selE", [1, 256], BC_DT, kind="ExternalInput").ap(),
        "out": nc.dram_tensor("out", [N, C], F32, kind="ExternalOutput").ap(),
    }
    import contextlib

    with tile.TileContext(nc) as tc:
        with contextlib.ExitStack() as ctx:
            _emit(tc, d, ctx)
    nc.finalize()
    return nc


def _mm_np_dtype():
    if MM_DT == mybir.dt.bfloat16:
        import ml_dtypes

        return ml_dtypes.bfloat16
    return np.float32


def _prep_host(x, q_w, q_b, kv_w, kv_b, proj_w, proj_b):
    """Transpose/pack on host. Returns (per-core xT list, shared map)."""
    f32 = np.float32
    x = np.asarray(x, f32)
    xT = np.ascontiguousarray(x.reshape(B, N, C).transpose(0, 2, 1))  # [B, C, N]

    qwT = np.ascontiguousarray(np.asarray(q_w, f32).T)  # [Cin, Cout]
    kwT = np.ascontiguousarray(np.asarray(kv_w[:C], f32).T)
    vwT = np.ascontiguousarray(np.asarray(kv_w[C:], f32).T)
    pwT = np.ascontiguousarray(np.asarray(proj_w, f32).T)

    wqP = np.zeros((C, PAIRS * P), f32)
    wkP = np.zeros((C, PAIRS * P), f32)
    qbP = np.zeros((P, PAIRS), f32)
    kbP = np.zeros((P, PAIRS), f32)
    pwP = np.zeros((PAIRS, P, C), f32)
    for p in range(PAIRS):
        a, b = 2 * p, 2 * p + 1
        wqP[:, p * P : p * P + HD] = qwT[:, a * HD : (a + 1) * HD]
        wqP[:, p * P + 64 : p * P + 64 + HD] = qwT[:, b * HD : (b + 1) * HD]
        wkP[:, p * P : p * P + HD] = kwT[:, a * HD : (a + 1) * HD]
        wkP[:, p * P + 64 : p * P + 64 + HD] = kwT[:, b * HD : (b + 1) * HD]
        qbP[0:HD, p] = q_b[a * HD : (a + 1) * HD]
        qbP[64 : 64 + HD, p] = q_b[b * HD : (b + 1) * HD]
        kbP[0:HD, p] = kv_b[a * HD : (a + 1) * HD]
        kbP[64 : 64 + HD, p] = kv_b[b * HD : (b + 1) * HD]
        # rows 1..48 / 65..112 carry the proj weights; rows 0 / 64 stay zero
        # to swallow the denominator row of outT.
        pwP[p, 1 : 1 + HD, :] = pwT[a * HD : (a + 1) * HD, :]
        pwP[p, 65 : 65 + HD, :] = pwT[b * HD : (b + 1) * HD, :]

    # V blocks are [ones | v0..v47] per head so the softmax denominator lands
    # at a 32-aligned PSUM partition (0 / 64).
    wvA = np.zeros((C, VW), f32)
    vbA = np.zeros((1, VW), f32)
    for h in range(NH):
        wvA[:, h * (HD + 1) + 1 : (h + 1) * (HD + 1)] = vwT[:, h * HD : (h + 1) * HD]
        vbA[0, h * (HD + 1) + 1 : (h + 1) * (HD + 1)] = kv_b[
            C + h * HD : C + (h + 1) * HD
        ]
        vbA[0, h * (HD + 1)] = 1.0

    selE = np.zeros((1, 256), f32)
    selE[0, 0 : HD + 1] = 1.0
    selE[0, 128 + 64 : 128 + 64 + HD + 1] = 1.0

    mmdt = _mm_np_dtype()
    shared = {
        "selE": selE,
        "wqP": wqP.astype(mmdt),
        "wkP": wkP.astype(mmdt),
        "wvA": wvA.astype(mmdt),
        "vbA": vbA.astype(mmdt),
        "qbP": qbP,
        "kbP": kbP,
        "pwP": pwP.astype(mmdt),
        "pbR": np.asarray(proj_b, f32).reshape(1, C).astype(mmdt),
    }
    return xT.astype(mmdt), shared


_PROGRAM = None


def _get_program():
    global _PROGRAM
    if _PROGRAM is None:
        _PROGRAM = build_program(B)
    return _PROGRAM


def kernel(x, q_w, q_b, kv_w, kv_b, proj_w, proj_b):
    xT, shared = _prep_host(x, q_w, q_b, kv_w, kv_b, proj_w, proj_b)
    nc = _get_program()
    in_maps = [dict(shared, xT=np.ascontiguousarray(xT[b])) for b in range(B)]
    res = run_bass_kernel_spmd(nc, in_maps, list(range(B)))
    outs = [np.asarray(res.results[i]["out"], np.float32) for i in range(B)]
    return np.stack(outs).reshape(B, HH, WW, C)
